# revision 1
# baseline (speedup 1.0000x reference)
"""Trainium2 Bass kernel for DampedAttention.

Full inputs in, full output out. Sharding: 8 cores = 2 batches x 4 head-groups
(4 heads of dim 64 each per core). Per core:

  QT/KT  [c, s] transposed projections (c on partitions), scale 1/8 and biases
         folded in (bias via K=1 ones-row matmuls, scale into weights on host)
  V      [s, c] natural projection (lhsT for the P@V matmul)
  ST     scores transposed [k, q] per (k-chunk, q-block) so exp(ST) is directly
         the lhsT-layout P^T needed by P@V -- no on-chip transposes
  ctxT   [65, q] = V_aug^T @ P^T ; row 64 = softmax row-sums (ones column in V)
  LVT    [64, q] banded 0.4*L^T matmuls (8 unique host-built band tiles)
  blend  ctxT_final = PV * (0.6/r, bcast over partitions) + LVT
  out    [s, o] natural out-projection; host sums 4 head-group partials + bo

Matmul operands are bf16 (fp32 matmul lowers to a 2x HI/LO instruction pair on
TRN2); accumulation, softmax row-sums, reciprocal and the 0.6/r normalization
stay fp32. The entropy gate in the reference is a forward no-op and is
skipped. Softmax max-subtraction is skipped (scores are O(1), no overflow).
"""
import numpy as np
import ml_dtypes

S = 2048
D = 1024
CLOC = 256          # channels per core (4 heads x 64)
HD = 64
NH = 4              # heads per core
NDC = 8             # 128-wide d-chunks in contraction D
NKC = 16            # 128-wide k/s chunks in S
NQB = 4             # 512-wide q blocks
QB = 512
WINDOW = 3
STRENGTH = 0.4
EPS = 1e-10
F32 = np.float32
BF16 = ml_dtypes.bfloat16


def _build_L04T():
    i = np.arange(S)
    d = (i[:, None] - i[None, :]).astype(F32)
    k = np.where(np.abs(d) <= WINDOW,
                 np.exp(-(d ** 2) / F32(2.0 * STRENGTH ** 2)),
                 F32(0.0)).astype(F32)
    L = k / (k.sum(axis=-1, keepdims=True) + F32(EPS))
    return (F32(0.4) * L).T.copy()  # [s, q], pre-scaled by (1 - lambda_jump)


def _lt_tiles():
    """Unique [128, 512] band tiles of 0.4*L^T plus (qb -> [(j, uniq_idx)])."""
    L04T = _build_L04T()
    uniq = []
    slots = {qb: [] for qb in range(NQB)}
    for qb in range(NQB):
        for j in range(max(0, qb * 4 - 1), min(NKC, qb * 4 + 5)):
            t = L04T[j * 128:(j + 1) * 128, qb * QB:(qb + 1) * QB]
            for ui, ut in enumerate(uniq):
                if np.array_equal(t, ut):
                    slots[qb].append((j, ui))
                    break
            else:
                slots[qb].append((j, len(uniq)))
                uniq.append(t)
    return np.stack(uniq).astype(BF16), slots


_LT_UNIQ, _LT_SLOTS = _lt_tiles()
NU = _LT_UNIQ.shape[0]

_CACHE = {}


def _build_program():
    import concourse.bacc as bacc
    import concourse.mybir as mybir
    from concourse.tile import TileContext
    from concourse.bass_isa import ReduceOp  # noqa: F401  (engine availability)

    f32 = mybir.dt.float32
    bf16 = mybir.dt.bfloat16
    Exp = mybir.ActivationFunctionType.Exp

    nc = bacc.Bacc("TRN2", target_bir_lowering=False, debug=False,
                   enable_asserts=False, num_devices=8)

    xt = nc.dram_tensor("xt", [D, S], bf16, kind="ExternalInput").ap()
    wqt = nc.dram_tensor("wqt", [D, CLOC], bf16, kind="ExternalInput").ap()
    wkt = nc.dram_tensor("wkt", [D, CLOC], bf16, kind="ExternalInput").ap()
    wvt = nc.dram_tensor("wvt", [D, CLOC], bf16, kind="ExternalInput").ap()
    bqr = nc.dram_tensor("bqr", [1, CLOC], bf16, kind="ExternalInput").ap()
    bkr = nc.dram_tensor("bkr", [1, CLOC], bf16, kind="ExternalInput").ap()
    bvr = nc.dram_tensor("bvr", [1, CLOC], bf16, kind="ExternalInput").ap()
    wot = nc.dram_tensor("wot", [CLOC, D], bf16, kind="ExternalInput").ap()
    ltt = nc.dram_tensor("ltt", [NU, 128, QB], bf16, kind="ExternalInput").ap()
    out = nc.dram_tensor("out", [S, D], f32, kind="ExternalOutput").ap()

    with TileContext(nc) as tc:
        with tc.tile_pool(name="persist", bufs=1) as pp:
            # ---- persistent SBUF ----
            # per-c-tile tensors so head-pair 0 attention is not
            # dependency-gated on c-tile 1 projections
            qt = [pp.tile([128, S], bf16, name=f"qt{i}") for i in range(2)]
            kt = [pp.tile([128, S], bf16, name=f"kt{i}") for i in range(2)]
            v_all = pp.tile([128, NKC, NH, HD + 1], bf16)  # ones col at 64
            ctxt_all = pp.tile([128, 2, S], bf16)
            wot_sb = pp.tile([128, 2, D], bf16)
            bq_sb = pp.tile([1, CLOC], bf16)
            bk_sb = pp.tile([1, CLOC], bf16)
            bv_sb = pp.tile([1, CLOC], bf16)
            ones_r = pp.tile([1, QB], bf16)          # ones row (bias outer prod)
            ones_c = pp.tile([1, 128], bf16)         # ones row (V bias)

            nc.gpsimd.memset(ones_r[:], 1.0)
            nc.gpsimd.memset(ones_c[:], 1.0)
            nc.gpsimd.memset(v_all[:, :, :, HD:HD + 1], 1.0)

            nc.sync.dma_start(bq_sb[:], bqr[:])
            nc.sync.dma_start(bk_sb[:], bkr[:])
            nc.sync.dma_start(bv_sb[:], bvr[:])
            for cc in range(2):
                nc.sync.dma_start(wot_sb[:, cc, :], wot[cc * 128:(cc + 1) * 128, :])

            # ---- phase B: projections ----
            with (
                tc.tile_pool(name="projsb", bufs=1) as prs,
                tc.tile_pool(name="projps", bufs=4, space="PSUM") as prp,
                tc.tile_pool(name="vps", bufs=2, space="PSUM") as vpp,
            ):
                xt_sb = prs.tile([128, NDC, S], bf16)
                for dc in range(NDC):
                    nc.sync.dma_start(xt_sb[:, dc, :],
                                      xt[dc * 128:(dc + 1) * 128, :])
                wq_sb = prs.tile([128, NDC, CLOC], bf16)
                wk_sb = prs.tile([128, NDC, CLOC], bf16)
                wv_sb = prs.tile([128, NDC, CLOC], bf16)
                for dc in range(NDC):
                    nc.sync.dma_start(wq_sb[:, dc, :], wqt[dc * 128:(dc + 1) * 128, :])
                    nc.sync.dma_start(wk_sb[:, dc, :], wkt[dc * 128:(dc + 1) * 128, :])
                    nc.sync.dma_start(wv_sb[:, dc, :], wvt[dc * 128:(dc + 1) * 128, :])

                # V natural first (attention needs all of V before any P@V):
                # [s-chunk 128, 256], contraction over d
                for sc in range(NKC):
                    ps = vpp.tile([128, CLOC], f32, tag="vps")
                    for dc in range(NDC):
                        nc.tensor.matmul(
                            ps[:],
                            xt_sb[:, dc, sc * 128:(sc + 1) * 128],
                            wv_sb[:, dc, :],
                            start=(dc == 0), stop=False)
                    nc.tensor.matmul(ps[:], ones_c[:], bv_sb[:],
                                     start=False, stop=True)
                    nc.vector.tensor_copy(
                        v_all[:, sc, :, 0:HD],
                        ps[:].rearrange("p (h e) -> p h e", h=NH))

                # QT / KT: [c-tile 128, s-block 512], contraction over d.
                # ct outermost so head-pair 0 attention can start after ct 0;
                # qb innermost so one weight load serves 4 matmuls.
                for ct in range(2):
                    for dst, w_sb, b_sb in ((qt[ct], wq_sb, bq_sb),
                                            (kt[ct], wk_sb, bk_sb)):
                        pss = [prp.tile([128, QB], f32, tag="projps", name=f"pjps{qb}")
                               for qb in range(NQB)]
                        for dc in range(NDC):
                            for qb in range(NQB):
                                nc.tensor.matmul(
                                    pss[qb][:],
                                    w_sb[:, dc, ct * 128:(ct + 1) * 128],
                                    xt_sb[:, dc, qb * QB:(qb + 1) * QB],
                                    start=(dc == 0), stop=False)
                        for qb in range(NQB):
                            nc.tensor.matmul(
                                pss[qb][:], b_sb[:, ct * 128:(ct + 1) * 128],
                                ones_r[:], start=False, stop=True)
                            nc.vector.tensor_copy(
                                dst[:, qb * QB:(qb + 1) * QB], pss[qb][:])

            # ---- phase C: attention per (head-pair, q-block) ----
            # Heads 2hp/2hp+1 live at partitions 0-63/64-127 of c-tile hp, so
            # interleaving their score matmuls alternates PE row-groups
            # (weight loads overlap compute) and keeps PE dense for HAM.
            with (
                tc.tile_pool(name="attnsb", bufs=1) as ab,
                tc.tile_pool(name="stage", bufs=2) as sp,
                tc.tile_pool(name="pt", bufs=6) as ptp,
                tc.tile_pool(name="stps", bufs=3, space="PSUM") as stp,
                tc.tile_pool(name="ctxps", bufs=2, space="PSUM") as ctp,
            ):
                lt_sb = ab.tile([128, NU, QB], bf16)
                for u in range(NU):
                    nc.sync.dma_start(lt_sb[:, u, :], ltt[u, :, :])
                mult = mybir.AluOpType.mult
                add = mybir.AluOpType.add
                for hp in range(2):
                    for qb in range(NQB):
                        qsl = slice(qb * QB, (qb + 1) * QB)
                        ctx = [ctp.tile([128, QB], f32, tag="ctxps", name=f"ctx{hh}")
                               for hh in range(2)]
                        for kc in range(NKC):
                            st_ps = stp.tile([128, 2, QB], f32, tag="stps")
                            for hh in range(2):
                                p0 = hh * 64
                                # explicit tile_position: K=64 row-group
                                # packing so the head pair runs concurrently
                                nc.tensor.matmul(
                                    st_ps[:, hh, :],
                                    kt[hp][p0:p0 + 64, kc * 128:(kc + 1) * 128],
                                    qt[hp][p0:p0 + 64, qsl],
                                    start=True, stop=True,
                                    tile_position=(p0, 0))
                            pt_sb = ptp.tile([128, 2, QB], bf16, tag="pt")
                            nc.scalar.activation(pt_sb[:], st_ps[:], Exp)
                            for hh in range(2):
                                nc.tensor.matmul(
                                    ctx[hh][0:HD + 1, :],
                                    v_all[:, kc, 2 * hp + hh, 0:HD + 1],
                                    pt_sb[:, hh, :],
                                    start=(kc == 0), stop=(kc == NKC - 1))
                        slots = _LT_SLOTS[qb]
                        # banded 0.4*L^T term, both heads column-packed into
                        # one psum tile (col strips 0-1 / 2-3 run concurrently)
                        lv_ps = stp.tile([128, QB], f32, tag="stps")
                        for n, (j, u) in enumerate(slots):
                            for hh in range(2):
                                nc.tensor.matmul(
                                    lv_ps[hh * HD:(hh + 1) * HD, :],
                                    v_all[:, j, 2 * hp + hh, 0:HD],
                                    lt_sb[:, u, :],
                                    start=(n == 0), stop=(n == len(slots) - 1),
                                    tile_position=(0, hh * HD),
                                    skip_group_check=True)
                        for hh in range(2):
                            h = 2 * hp + hh
                            # 1/rowsum = exp(-ln r) on ScalarE (~2x faster than
                            # the DVE reciprocal; Ln and Exp share a table set)
                            lnr = sp.tile([65, QB], f32, tag="lnr")
                            nc.scalar.activation(
                                lnr[64:65, :], ctx[hh][64:65, :],
                                mybir.ActivationFunctionType.Ln)
                            bc_src = sp.tile([1, QB], f32, tag="bcsrc")
                            nc.scalar.activation(
                                bc_src[0:1, :], lnr[64:65, :],
                                mybir.ActivationFunctionType.Exp, scale=-1.0)
                            bc_sb = sp.tile([64, QB], f32, tag="bcsb")
                            nc.gpsimd.partition_broadcast(
                                bc_sb[:], bc_src[:], channels=HD)
                            # blend: (PV/r)*0.6 + 0.4LV, staged out per q-block
                            m1 = sp.tile([64, QB], f32, tag="m1")
                            nc.vector.tensor_mul(m1[:], ctx[hh][0:HD, :], bc_sb[:])
                            stg = sp.tile([64, QB], bf16, tag="stg")
                            nc.vector.scalar_tensor_tensor(
                                stg[:], m1[:], 0.6,
                                lv_ps[hh * HD:(hh + 1) * HD, :],
                                op0=mult, op1=add)
                            nc.sync.dma_start(
                                ctxt_all[hh * 64:hh * 64 + 64, hp, qsl], stg[:])

            # ---- phase D: out-projection ----
            with (
                tc.tile_pool(name="ops", bufs=2, space="PSUM") as opp,
                tc.tile_pool(name="osb", bufs=4) as osb,
            ):
                for sc in range(NKC):
                    for ot in range(2):
                        ps = opp.tile([128, QB], f32, tag="ops")
                        for cc in range(2):
                            nc.tensor.matmul(
                                ps[:],
                                ctxt_all[:, cc, sc * 128:(sc + 1) * 128],
                                wot_sb[:, cc, ot * QB:(ot + 1) * QB],
                                start=(cc == 0), stop=(cc == 1))
                        ot_sb = osb.tile([128, QB], f32, tag="osb")
                        nc.vector.tensor_copy(ot_sb[:], ps[:])
                        nc.sync.dma_start(
                            out[sc * 128:(sc + 1) * 128, ot * QB:(ot + 1) * QB],
                            ot_sb[:])

    nc.compile()
    return nc


def _get_program():
    if "nc" not in _CACHE:
        _CACHE["nc"] = _build_program()
    return _CACHE["nc"]


def _in_maps(x, Wq, bq, Wk, bk, Wv, bv, Wo):
    xT = [np.ascontiguousarray(x[b].T).astype(BF16) for b in range(2)]
    maps = []
    for c in range(8):
        b, hg = c // 4, c % 4
        hs, he = hg * CLOC, (hg + 1) * CLOC
        maps.append({
            "xt": xT[b],
            "wqt": np.ascontiguousarray(Wq[hs:he].T / F32(8.0)).astype(BF16),
            "wkt": np.ascontiguousarray(Wk[hs:he].T).astype(BF16),
            "wvt": np.ascontiguousarray(Wv[hs:he].T).astype(BF16),
            "bqr": (bq[hs:he] / F32(8.0))[None, :].astype(BF16),
            "bkr": bk[hs:he][None, :].astype(BF16),
            "bvr": bv[hs:he][None, :].astype(BF16),
            "wot": np.ascontiguousarray(Wo[:, hs:he].T).astype(BF16),
            "ltt": _LT_UNIQ,
        })
    return maps


def _run(x, Wq, bq, Wk, bk, Wv, bv, Wo, bo, trace=False):
    from concourse.bass_utils import run_bass_kernel_spmd
    nc = _get_program()
    maps = _in_maps(np.asarray(x, F32), np.asarray(Wq, F32), np.asarray(bq, F32),
                    np.asarray(Wk, F32), np.asarray(bk, F32), np.asarray(Wv, F32),
                    np.asarray(bv, F32), np.asarray(Wo, F32))
    res = run_bass_kernel_spmd(nc, maps, list(range(8)), trace=trace)
    bo = np.asarray(bo, F32)
    outp = np.empty((2, S, D), F32)
    for b in range(2):
        acc = res.results[b * 4]["out"].astype(F32)
        for hg in range(1, 4):
            acc = acc + res.results[b * 4 + hg]["out"]
        outp[b] = acc + bo
    return outp, res


def kernel(x, Wq, bq, Wk, bk, Wv, bv, Wo, bo):
    outp, _ = _run(x, Wq, bq, Wk, bk, Wv, bv, Wo, bo, trace=False)
    return outp


def kernel_traced(**inputs):
    return _run(trace=True, **inputs)



# revision 6
# speedup vs baseline: 1.0057x; 1.0057x over previous
"""Trainium2 Bass kernel for DampedAttention.

Full inputs in, full output out. Sharding: 8 cores = 2 batches x 4 head-groups
(4 heads of dim 64 each per core). Per core:

  QT/KT  [c, s] transposed projections (c on partitions), scale 1/8 folded
         into wq/bq on host; bias folded via per-partition activation bias
         on the PSUM->SBUF copy (ScalarE, idle during projections)
  V      [s, c] natural projection (lhsT for the P@V matmul); bias via a
         K=1 ones-row matmul
  LV     banded 0.4*L^T term precomputed per (hp, qb) into SBUF during the
         projection phase; both heads per matmul (lhsT = [V_h0|V_h1], M=128)
  ST     scores transposed [k, q] per (k-chunk, q-block) so exp(ST) is the
         lhsT-layout P^T needed by P@V -- no on-chip transposes
  exp    software-pipelined: scores(kc+2) and exp(kc+1) run ahead of pv(kc)
         so ScalarE (the bottleneck) streams exps back-to-back
  ctxT   [65, q] = V_aug^T @ P^T ; row 64 = softmax row-sums (ones column)
  blend  ctxT_final = PV * (0.6/r) + 0.4LV; 1/r via DVE reciprocal_approx
         + gpsimd partition broadcast (keeps ScalarE exp-only: one act table)
  out    [s, o] out-projection matmuls injected into later attention groups'
         loops as PE filler; host sums 4 head-group partials + bo

Matmul operands are bf16; accumulation, row-sums, reciprocal and the 0.6/r
normalization stay fp32. The entropy gate in the reference is a forward
no-op and is skipped. Softmax max-subtraction is skipped (scores are O(1)).
"""
import numpy as np
import ml_dtypes

S = 2048
D = 1024
CLOC = 256          # channels per core (4 heads x 64)
HD = 64
NH = 4              # heads per core
NDC = 8             # 128-wide d-chunks in contraction D
NKC = 16            # 128-wide k/s chunks in S
NQB = 4             # 512-wide q blocks
QB = 512
WINDOW = 3
STRENGTH = 0.4
EPS = 1e-10
F32 = np.float32
BF16 = ml_dtypes.bfloat16


def _build_L04T():
    i = np.arange(S)
    d = (i[:, None] - i[None, :]).astype(F32)
    k = np.where(np.abs(d) <= WINDOW,
                 np.exp(-(d ** 2) / F32(2.0 * STRENGTH ** 2)),
                 F32(0.0)).astype(F32)
    L = k / (k.sum(axis=-1, keepdims=True) + F32(EPS))
    return (F32(0.4) * L).T.copy()  # [s, q], pre-scaled by (1 - lambda_jump)


def _lt_tiles():
    """Unique [128, 512] band tiles of 0.4*L^T plus (qb -> [(j, uniq_idx)])."""
    L04T = _build_L04T()
    uniq = []
    slots = {qb: [] for qb in range(NQB)}
    for qb in range(NQB):
        for j in range(max(0, qb * 4 - 1), min(NKC, qb * 4 + 5)):
            t = L04T[j * 128:(j + 1) * 128, qb * QB:(qb + 1) * QB]
            for ui, ut in enumerate(uniq):
                if np.array_equal(t, ut):
                    slots[qb].append((j, ui))
                    break
            else:
                slots[qb].append((j, len(uniq)))
                uniq.append(t)
    return np.stack(uniq).astype(BF16), slots


_LT_UNIQ, _LT_SLOTS = _lt_tiles()
NU = _LT_UNIQ.shape[0]

_CACHE = {}


def _build_program():
    import concourse.bacc as bacc
    import concourse.mybir as mybir
    from concourse.tile import TileContext

    f32 = mybir.dt.float32
    bf16 = mybir.dt.bfloat16
    Exp = mybir.ActivationFunctionType.Exp
    Ident = mybir.ActivationFunctionType.Identity
    mult = mybir.AluOpType.mult
    add = mybir.AluOpType.add

    nc = bacc.Bacc("TRN2", target_bir_lowering=False, debug=False,
                   enable_asserts=False, num_devices=8)

    xt = nc.dram_tensor("xt", [D, S], bf16, kind="ExternalInput").ap()
    wqt = nc.dram_tensor("wqt", [D, CLOC], bf16, kind="ExternalInput").ap()
    wkt = nc.dram_tensor("wkt", [D, CLOC], bf16, kind="ExternalInput").ap()
    wvt = nc.dram_tensor("wvt", [D, CLOC], bf16, kind="ExternalInput").ap()
    bqc = nc.dram_tensor("bqc", [CLOC, 1], f32, kind="ExternalInput").ap()
    bkc = nc.dram_tensor("bkc", [CLOC, 1], f32, kind="ExternalInput").ap()
    bvr = nc.dram_tensor("bvr", [1, CLOC], bf16, kind="ExternalInput").ap()
    wot = nc.dram_tensor("wot", [CLOC, D], bf16, kind="ExternalInput").ap()
    ltt = nc.dram_tensor("ltt", [NU, 128, QB], bf16, kind="ExternalInput").ap()
    out = nc.dram_tensor("out", [S, D], f32, kind="ExternalOutput").ap()

    with TileContext(nc) as tc:
        with tc.tile_pool(name="persist", bufs=1) as pp:
            # ---- persistent SBUF ----
            qt = [pp.tile([128, S], bf16, name=f"qt{i}") for i in range(2)]
            kt = [pp.tile([128, S], bf16, name=f"kt{i}") for i in range(2)]
            v_all = pp.tile([128, NKC, NH, HD + 1], bf16)  # ones col at 64
            v_pair = pp.tile([128, NKC, CLOC], bf16)  # contiguous, no ones col
            lv_sb = pp.tile([128, 2, S], bf16)   # rows 0-63 hh0 / 64-127 hh1
            lv1_sb = pp.tile([64, 2, S], bf16)   # hh1 band term at rows 0-63
            ctxt_all = pp.tile([128, 2, S], bf16)
            wot_sb = pp.tile([128, 2, D], bf16)
            bq_sb = pp.tile([128, 2], f32)       # per-partition bias columns
            bk_sb = pp.tile([128, 2], f32)
            bv_sb = pp.tile([1, CLOC], bf16)
            lt_sb = pp.tile([128, NU, QB], bf16)
            ones_c = pp.tile([1, 128], bf16)     # ones row (V bias)

            nc.gpsimd.memset(ones_c[:], 1.0)
            nc.gpsimd.memset(v_all[:, :, :, HD:HD + 1], 1.0)

            for ct in range(2):
                nc.sync.dma_start(bq_sb[:, ct:ct + 1],
                                  bqc[ct * 128:(ct + 1) * 128, :])
                nc.sync.dma_start(bk_sb[:, ct:ct + 1],
                                  bkc[ct * 128:(ct + 1) * 128, :])
                nc.sync.dma_start(wot_sb[:, ct, :], wot[ct * 128:(ct + 1) * 128, :])
            nc.sync.dma_start(bv_sb[:], bvr[:])
            for u in range(NU):
                nc.sync.dma_start(lt_sb[:, u, :], ltt[u, :, :])

            # ---- phase B: projections + LV ----
            with (
                tc.tile_pool(name="projsb", bufs=1) as prs,
                tc.tile_pool(name="projps", bufs=4, space="PSUM") as prp,
                tc.tile_pool(name="vps", bufs=2, space="PSUM") as vpp,
                tc.tile_pool(name="lvps", bufs=2, space="PSUM") as lvp,
            ):
                xt_sb = prs.tile([128, NDC, S], bf16)
                wq_sb = prs.tile([128, NDC, CLOC], bf16)
                wk_sb = prs.tile([128, NDC, CLOC], bf16)
                wv_sb = prs.tile([128, NDC, CLOC], bf16)
                # dc-major DMA: first QK matmul (dc=0) can start after ~1.7us
                for dc in range(NDC):
                    nc.sync.dma_start(wq_sb[:, dc, :], wqt[dc * 128:(dc + 1) * 128, :])
                    nc.sync.dma_start(wk_sb[:, dc, :], wkt[dc * 128:(dc + 1) * 128, :])
                    for qb in range(NQB):
                        nc.sync.dma_start(
                            xt_sb[:, dc, qb * QB:(qb + 1) * QB],
                            xt[dc * 128:(dc + 1) * 128, qb * QB:(qb + 1) * QB])
                for dc in range(NDC):
                    nc.sync.dma_start(wv_sb[:, dc, :], wvt[dc * 128:(dc + 1) * 128, :])

                # QT/KT: [c-tile 128, s-block 512], contraction over d.
                # dc outermost so one weight load serves 4 qb matmuls; bias
                # added on the PSUM->SBUF copy (ScalarE, per-partition bias).
                for ct in range(2):
                    for dst, w_sb, b_sb in ((qt[ct], wq_sb, bq_sb),
                                            (kt[ct], wk_sb, bk_sb)):
                        pss = [prp.tile([128, QB], f32, tag="projps",
                                        name=f"pjps{qb}") for qb in range(NQB)]
                        for dc in range(NDC):
                            for qb in range(NQB):
                                nc.tensor.matmul(
                                    pss[qb][:],
                                    w_sb[:, dc, ct * 128:(ct + 1) * 128],
                                    xt_sb[:, dc, qb * QB:(qb + 1) * QB],
                                    start=(dc == 0), stop=(dc == NDC - 1))
                        for qb in range(NQB):
                            nc.scalar.activation(
                                dst[:, qb * QB:(qb + 1) * QB], pss[qb][:],
                                Ident, bias=b_sb[:, ct:ct + 1])

                # V natural: [s-chunk 128, 256], contraction over d
                for sc in range(NKC):
                    ps = vpp.tile([128, CLOC], f32, tag="vps")
                    for dc in range(NDC):
                        nc.tensor.matmul(
                            ps[:],
                            xt_sb[:, dc, sc * 128:(sc + 1) * 128],
                            wv_sb[:, dc, :],
                            start=(dc == 0), stop=False)
                    nc.tensor.matmul(ps[:], ones_c[:], bv_sb[:],
                                     start=False, stop=True)
                    nc.vector.tensor_copy(
                        v_all[:, sc, :, 0:HD],
                        ps[:].rearrange("p (h e) -> p h e", h=NH))
                    nc.vector.tensor_copy(v_pair[:, sc, :], ps[:])

                # LV: banded 0.4*L^T @ V, both heads per matmul (M=128)
                for hp in range(2):
                    for qb in range(NQB):
                        slots = _LT_SLOTS[qb]
                        lv_ps = lvp.tile([128, QB], f32, tag="lvps")
                        for n, (j, u) in enumerate(slots):
                            nc.tensor.matmul(
                                lv_ps[:],
                                v_pair[:, j, hp * 128:(hp + 1) * 128],
                                lt_sb[:, u, :],
                                start=(n == 0), stop=(n == len(slots) - 1))
                        nc.vector.tensor_copy(
                            lv_sb[:, hp, qb * QB:(qb + 1) * QB], lv_ps[:])
                for hp in range(2):
                    nc.sync.dma_start(lv1_sb[:, hp, :], lv_sb[64:128, hp, :])

            # ---- phase C: attention, software-pipelined ----
            # Heads 2hp/2hp+1 live at partitions 0-63/64-127 of c-tile hp.
            # Per (qb, hp) group: scores(kc+2)/exp(kc+1) run ahead of pv(kc)
            # so ScalarE (exp, the bottleneck) streams back-to-back while PE
            # fills its spare cycles with injected out-projection matmuls.
            with (
                tc.tile_pool(name="pt", bufs=4) as ptp,
                tc.tile_pool(name="stage", bufs=4) as sp,
                tc.tile_pool(name="osb", bufs=4) as ob,
                tc.tile_pool(name="stps", bufs=2, space="PSUM") as stp,
                tc.tile_pool(name="ctxps", bufs=3, space="PSUM") as ctp,
                tc.tile_pool(name="ops", bufs=1, space="PSUM") as opp,
            ):
                def emit_outproj(sc, ot, pool, sbpool):
                    ps = pool.tile([128, QB], f32, tag="ops")
                    for cc in range(2):
                        nc.tensor.matmul(
                            ps[:],
                            ctxt_all[:, cc, sc * 128:(sc + 1) * 128],
                            wot_sb[:, cc, ot * QB:(ot + 1) * QB],
                            start=(cc == 0), stop=(cc == 1),
                            skip_group_check=True)
                    o_sb = sbpool.tile([128, QB], f32, tag="osb")
                    nc.vector.tensor_copy(o_sb[:], ps[:])
                    nc.sync.dma_start(
                        out[sc * 128:(sc + 1) * 128, ot * QB:(ot + 1) * QB],
                        o_sb[:])

                filler = []

                def emit_group(qb, hp):
                    qsl = slice(qb * QB, (qb + 1) * QB)
                    ctx = [ctp.tile([128, QB], f32, tag="ctxps",
                                    name=f"ctx{hh}") for hh in range(2)]
                    sts, pts = {}, {}

                    def scores(kc):
                        st = stp.tile([128, 2, QB], f32, tag="stps")
                        sts[kc] = st
                        for hh in range(2):
                            p0 = hh * 64
                            nc.tensor.matmul(
                                st[:, hh, :],
                                kt[hp][p0:p0 + 64, kc * 128:(kc + 1) * 128],
                                qt[hp][p0:p0 + 64, qsl],
                                start=True, stop=True,
                                tile_position=(p0, 0))
                        ptt = ptp.tile([128, 2, QB], bf16, tag="pt")
                        pts[kc] = ptt
                        nc.scalar.activation(ptt[:], st[:], Exp)

                    def pv(kc):
                        ptt = pts.pop(kc)
                        sts.pop(kc)
                        for hh in range(2):
                            nc.tensor.matmul(
                                ctx[hh][0:HD + 1, :],
                                v_all[:, kc, 2 * hp + hh, 0:HD + 1],
                                ptt[:, hh, :],
                                start=(kc == 0), stop=(kc == NKC - 1))

                    scores(0)
                    scores(1)
                    for kc in range(NKC):
                        if kc + 2 < NKC:
                            scores(kc + 2)
                        pv(kc)
                        if kc in (4, 7, 10, 13) and filler:
                            sc, ot = filler.pop(0)
                            emit_outproj(sc, ot, opp, ob)

                    # blend: ctxt = (PV * 0.6/r) + 0.4LV
                    for hh in range(2):
                        # partition 64 -> 0 move on ScalarE (tiny; Copy needs
                        # no act table so the Exp table stays resident)
                        bcs = sp.tile([1, QB], f32, tag="bcs")
                        nc.scalar.activation(
                            bcs[0:1, :], ctx[hh][64:65, :],
                            mybir.ActivationFunctionType.Copy)
                        rc = sp.tile([1, QB], f32, tag="rc")
                        nc.vector.reciprocal(rc[:], bcs[:])
                        bc = sp.tile([64, QB], f32, tag="bc")
                        nc.gpsimd.partition_broadcast(
                            bc[:], rc[0:1, :], channels=HD)
                        m1 = sp.tile([64, QB], f32, tag="m1")
                        nc.vector.tensor_mul(m1[:], ctx[hh][0:HD, :], bc[:])
                        if hh == 0:
                            nc.vector.scalar_tensor_tensor(
                                ctxt_all[0:64, hp, qsl], m1[:], 0.6,
                                lv_sb[0:64, hp, qsl], op0=mult, op1=add)
                        else:
                            stg = sp.tile([64, QB], bf16, tag="stg")
                            nc.vector.scalar_tensor_tensor(
                                stg[:], m1[:], 0.6,
                                lv1_sb[:, hp, qsl], op0=mult, op1=add)
                            nc.sync.dma_start(
                                ctxt_all[64:128, hp, qsl], stg[:])

                # (qb, hp) order chosen so out-proj(qb) becomes injectable
                # 1-2 groups after both hp halves of qb have blended.
                groups = [(0, 0), (1, 0), (0, 1), (1, 1),
                          (2, 0), (3, 0), (2, 1), (3, 1)]
                push_after = {2: 0, 3: 1, 6: 2, 7: 3}
                for gi, (qb, hp) in enumerate(groups):
                    emit_group(qb, hp)
                    if gi in push_after:
                        qb_o = push_after[gi]
                        for sc in range(qb_o * 4, qb_o * 4 + 4):
                            for ot in range(2):
                                filler.append((sc, ot))

            # ---- phase D: out-projection tail (wider PSUM pool) ----
            with (
                tc.tile_pool(name="ops2", bufs=4, space="PSUM") as opp2,
                tc.tile_pool(name="osb2", bufs=4) as ob2,
            ):
                while filler:
                    sc, ot = filler.pop(0)
                    emit_outproj(sc, ot, opp2, ob2)

    nc.compile()
    return nc


def _get_program():
    if "nc" not in _CACHE:
        _CACHE["nc"] = _build_program()
    return _CACHE["nc"]


def _in_maps(x, Wq, bq, Wk, bk, Wv, bv, Wo):
    xT = [np.ascontiguousarray(x[b].T).astype(BF16) for b in range(2)]
    maps = []
    for c in range(8):
        b, hg = c // 4, c % 4
        hs, he = hg * CLOC, (hg + 1) * CLOC
        maps.append({
            "xt": xT[b],
            "wqt": np.ascontiguousarray(Wq[hs:he].T / F32(8.0)).astype(BF16),
            "wkt": np.ascontiguousarray(Wk[hs:he].T).astype(BF16),
            "wvt": np.ascontiguousarray(Wv[hs:he].T).astype(BF16),
            "bqc": (bq[hs:he] / F32(8.0))[:, None].astype(F32),
            "bkc": bk[hs:he][:, None].astype(F32),
            "bvr": bv[hs:he][None, :].astype(BF16),
            "wot": np.ascontiguousarray(Wo[:, hs:he].T).astype(BF16),
            "ltt": _LT_UNIQ,
        })
    return maps


def _run(x, Wq, bq, Wk, bk, Wv, bv, Wo, bo, trace=False):
    from concourse.bass_utils import run_bass_kernel_spmd
    nc = _get_program()
    maps = _in_maps(np.asarray(x, F32), np.asarray(Wq, F32), np.asarray(bq, F32),
                    np.asarray(Wk, F32), np.asarray(bk, F32), np.asarray(Wv, F32),
                    np.asarray(bv, F32), np.asarray(Wo, F32))
    res = run_bass_kernel_spmd(nc, maps, list(range(8)), trace=trace)
    bo = np.asarray(bo, F32)
    outp = np.empty((2, S, D), F32)
    for b in range(2):
        acc = res.results[b * 4]["out"].astype(F32)
        for hg in range(1, 4):
            acc = acc + res.results[b * 4 + hg]["out"]
        outp[b] = acc + bo
    return outp, res


def kernel(x, Wq, bq, Wk, bk, Wv, bv, Wo, bo):
    outp, _ = _run(x, Wq, bq, Wk, bk, Wv, bv, Wo, bo, trace=False)
    return outp


def kernel_traced(**inputs):
    return _run(trace=True, **inputs)


# revision 11
# speedup vs baseline: 1.2460x; 1.2390x over previous
"""Trainium2 Bass kernel for DampedAttention.

Full inputs in, full output out. Sharding: 8 cores = 2 batches x 4 head-groups
(4 heads of dim 64 each per core). Per core:

  QT/KT  [c, s] transposed projections (c on partitions), scale 1/8 folded
         into wq/bq on host; bias folded via per-partition activation bias
         on the PSUM->SBUF copy (ScalarE, idle during projections)
  V      [s, c] natural projection (lhsT for the P@V matmul); bias via a
         K=1 ones-row matmul
  LV     banded 0.4*L^T term precomputed per (hp, qb) into SBUF during the
         projection phase; both heads per matmul (lhsT = [V_h0|V_h1], M=128)
  ST     scores transposed [k, q] per (k-chunk, q-block) so exp(ST) is the
         lhsT-layout P^T needed by P@V -- no on-chip transposes
  exp    software-pipelined: scores(kc+2) and exp(kc+1) run ahead of pv(kc)
         so ScalarE (the bottleneck) streams exps back-to-back
  ctxT   [65, q] = V_aug^T @ P^T ; row 64 = softmax row-sums (ones column)
  blend  ctxT_final = PV * (0.6/r) + 0.4LV; 1/r via DVE reciprocal_approx
         + gpsimd partition broadcast (keeps ScalarE exp-only: one act table)
  out    [s, o] out-projection matmuls injected into later attention groups'
         loops as PE filler; host sums 4 head-group partials + bo

Matmul operands are bf16; accumulation, row-sums, reciprocal and the 0.6/r
normalization stay fp32. The entropy gate in the reference is a forward
no-op and is skipped. Softmax max-subtraction is skipped (scores are O(1)).
"""
import numpy as np
import ml_dtypes

S = 2048
D = 1024
CLOC = 256          # channels per core (4 heads x 64)
HD = 64
NH = 4              # heads per core
NDC = 8             # 128-wide d-chunks in contraction D
NKC = 16            # 128-wide k/s chunks in S
NQB = 4             # 512-wide q blocks
QB = 512
WINDOW = 3
STRENGTH = 0.4
EPS = 1e-10
F32 = np.float32
BF16 = ml_dtypes.bfloat16


def _build_L04T():
    i = np.arange(S)
    d = (i[:, None] - i[None, :]).astype(F32)
    k = np.where(np.abs(d) <= WINDOW,
                 np.exp(-(d ** 2) / F32(2.0 * STRENGTH ** 2)),
                 F32(0.0)).astype(F32)
    L = k / (k.sum(axis=-1, keepdims=True) + F32(EPS))
    return (F32(0.4) * L).T.copy()  # [s, q], pre-scaled by (1 - lambda_jump)


def _lt_tiles():
    """Unique [128, 512] band tiles of 0.4*L^T plus (qb -> [(j, uniq_idx)])."""
    L04T = _build_L04T()
    uniq = []
    slots = {qb: [] for qb in range(NQB)}
    for qb in range(NQB):
        for j in range(max(0, qb * 4 - 1), min(NKC, qb * 4 + 5)):
            t = L04T[j * 128:(j + 1) * 128, qb * QB:(qb + 1) * QB]
            for ui, ut in enumerate(uniq):
                if np.array_equal(t, ut):
                    slots[qb].append((j, ui))
                    break
            else:
                slots[qb].append((j, len(uniq)))
                uniq.append(t)
    return np.stack(uniq).astype(BF16), slots


_LT_UNIQ, _LT_SLOTS = _lt_tiles()
NU = _LT_UNIQ.shape[0]

_CACHE = {}


def _build_program():
    import concourse.bacc as bacc
    import concourse.mybir as mybir
    from concourse.tile import TileContext

    f32 = mybir.dt.float32
    bf16 = mybir.dt.bfloat16
    Exp = mybir.ActivationFunctionType.Exp
    Ident = mybir.ActivationFunctionType.Identity
    mult = mybir.AluOpType.mult
    add = mybir.AluOpType.add

    nc = bacc.Bacc("TRN2", target_bir_lowering=False, debug=False,
                   enable_asserts=False, num_devices=8)

    xt = nc.dram_tensor("xt", [D, S], bf16, kind="ExternalInput").ap()
    wqt = nc.dram_tensor("wqt", [D, CLOC], bf16, kind="ExternalInput").ap()
    wkt = nc.dram_tensor("wkt", [D, CLOC], bf16, kind="ExternalInput").ap()
    wvt = nc.dram_tensor("wvt", [D, CLOC], bf16, kind="ExternalInput").ap()
    bqc = nc.dram_tensor("bqc", [128, 2], f32, kind="ExternalInput").ap()
    bkc = nc.dram_tensor("bkc", [128, 2], f32, kind="ExternalInput").ap()
    bvr = nc.dram_tensor("bvr", [1, CLOC], bf16, kind="ExternalInput").ap()
    wot = nc.dram_tensor("wot", [CLOC, D], bf16, kind="ExternalInput").ap()
    ltt = nc.dram_tensor("ltt", [NU, 128, QB], bf16, kind="ExternalInput").ap()
    out = nc.dram_tensor("out", [S, D], f32, kind="ExternalOutput").ap()

    with TileContext(nc) as tc:
        with tc.tile_pool(name="persist", bufs=1) as pp:
            # ---- persistent SBUF ----
            qt = [pp.tile([128, S], bf16, name=f"qt{i}") for i in range(2)]
            kt = [pp.tile([128, S], bf16, name=f"kt{i}") for i in range(2)]
            v_all = pp.tile([128, NKC, NH, HD + 1], bf16)  # ones col at 64
            v_pair = pp.tile([128, NKC, CLOC], bf16)  # contiguous, no ones col
            lv_sb = pp.tile([128, 2, S], bf16)   # rows 0-63 hh0 / 64-127 hh1
            lv1_sb = pp.tile([64, 2, S], bf16)   # hh1 band term at rows 0-63
            ctxt_all = pp.tile([128, 2, S], bf16)
            wot_sb = pp.tile([128, 2, D], bf16)
            bq_sb = pp.tile([128, 2], f32)       # per-partition bias columns
            bk_sb = pp.tile([128, 2], f32)
            bv_sb = pp.tile([1, CLOC], bf16)
            lt_sb = pp.tile([128, NU, QB], bf16)
            ones_c = pp.tile([1, 128], bf16)     # ones row (V bias)

            nc.gpsimd.memset(ones_c[:], 1.0)
            nc.gpsimd.memset(v_all[:, :, :, HD:HD + 1], 1.0)

            nc.sync.dma_start(bq_sb[:], bqc[:])
            nc.sync.dma_start(bk_sb[:], bkc[:])
            nc.sync.dma_start(bv_sb[:], bvr[:])
            nc.sync.dma_start(wot_sb[:, :, :],
                              wot.rearrange("(cc p) o -> p cc o", p=128))
            nc.sync.dma_start(lt_sb[:, :, :], ltt.rearrange("u p q -> p u q"))

            # ---- phase B: projections + LV ----
            with (
                tc.tile_pool(name="projsb", bufs=1) as prs,
                tc.tile_pool(name="projps", bufs=4, space="PSUM") as prp,
                tc.tile_pool(name="vps", bufs=2, space="PSUM") as vpp,
                tc.tile_pool(name="lvps", bufs=2, space="PSUM") as lvp,
            ):
                xt_sb = prs.tile([128, NDC, S], bf16)
                wq_sb = prs.tile([128, NDC, CLOC], bf16)
                wk_sb = prs.tile([128, NDC, CLOC], bf16)
                wv_sb = prs.tile([128, NDC, CLOC], bf16)
                # Few, large DMAs: each DMA instr costs ~600ns on the serial
                # Sync queue regardless of size, so instruction count rules.
                nc.sync.dma_start(wq_sb[:, :, :],
                                  wqt.rearrange("(dc p) c -> p dc c", p=128))
                nc.sync.dma_start(wk_sb[:, :, :],
                                  wkt.rearrange("(dc p) c -> p dc c", p=128))
                for dc in range(NDC):  # dc-granular so matmul dc=0 starts early
                    nc.sync.dma_start(xt_sb[:, dc, :],
                                      xt[dc * 128:(dc + 1) * 128, :])
                nc.sync.dma_start(wv_sb[:, :, :],
                                  wvt.rearrange("(dc p) c -> p dc c", p=128))

                # QT/KT: [c-tile 128, s-block 512], contraction over d.
                # dc outermost so one weight load serves 4 qb matmuls; bias
                # added on the PSUM->SBUF copy (ScalarE, per-partition bias).
                for ct in range(2):
                    for dst, w_sb, b_sb in ((qt[ct], wq_sb, bq_sb),
                                            (kt[ct], wk_sb, bk_sb)):
                        pss = [prp.tile([128, QB], f32, tag="projps",
                                        name=f"pjps{qb}") for qb in range(NQB)]
                        for dc in range(NDC):
                            for qb in range(NQB):
                                nc.tensor.matmul(
                                    pss[qb][:],
                                    w_sb[:, dc, ct * 128:(ct + 1) * 128],
                                    xt_sb[:, dc, qb * QB:(qb + 1) * QB],
                                    start=(dc == 0), stop=(dc == NDC - 1))
                        for qb in range(NQB):
                            nc.scalar.activation(
                                dst[:, qb * QB:(qb + 1) * QB], pss[qb][:],
                                Ident, bias=b_sb[:, ct:ct + 1])

                # V natural: [s-chunk 128, 256], contraction over d
                for sc in range(NKC):
                    ps = vpp.tile([128, CLOC], f32, tag="vps")
                    for dc in range(NDC):
                        nc.tensor.matmul(
                            ps[:],
                            xt_sb[:, dc, sc * 128:(sc + 1) * 128],
                            wv_sb[:, dc, :],
                            start=(dc == 0), stop=False)
                    nc.tensor.matmul(ps[:], ones_c[:], bv_sb[:],
                                     start=False, stop=True)
                    nc.vector.tensor_copy(
                        v_all[:, sc, :, 0:HD],
                        ps[:].rearrange("p (h e) -> p h e", h=NH))
                    nc.vector.tensor_copy(v_pair[:, sc, :], ps[:])

                # LV: banded 0.4*L^T @ V, both heads per matmul (M=128)
                for hp in range(2):
                    for qb in range(NQB):
                        slots = _LT_SLOTS[qb]
                        lv_ps = lvp.tile([128, QB], f32, tag="lvps")
                        for n, (j, u) in enumerate(slots):
                            nc.tensor.matmul(
                                lv_ps[:],
                                v_pair[:, j, hp * 128:(hp + 1) * 128],
                                lt_sb[:, u, :],
                                start=(n == 0), stop=(n == len(slots) - 1))
                        nc.vector.tensor_copy(
                            lv_sb[:, hp, qb * QB:(qb + 1) * QB], lv_ps[:])
                for hp in range(2):
                    nc.sync.dma_start(lv1_sb[:, hp, :], lv_sb[64:128, hp, :])

            # ---- phase C: attention, software-pipelined ----
            # Heads 2hp/2hp+1 live at partitions 0-63/64-127 of c-tile hp.
            # Per (qb, hp) group: scores(kc+2)/exp(kc+1) run ahead of pv(kc)
            # so ScalarE (exp, the bottleneck) streams back-to-back while PE
            # fills its spare cycles with injected out-projection matmuls.
            with (
                tc.tile_pool(name="pt", bufs=4) as ptp,
                tc.tile_pool(name="stage", bufs=4) as sp,
                tc.tile_pool(name="osb", bufs=4) as ob,
                tc.tile_pool(name="stps", bufs=2, space="PSUM") as stp,
                tc.tile_pool(name="ctxps", bufs=3, space="PSUM") as ctp,
                tc.tile_pool(name="ops", bufs=1, space="PSUM") as opp,
            ):
                def emit_outproj(sc, ot, pool, sbpool):
                    ps = pool.tile([128, QB], f32, tag="ops")
                    for cc in range(2):
                        nc.tensor.matmul(
                            ps[:],
                            ctxt_all[:, cc, sc * 128:(sc + 1) * 128],
                            wot_sb[:, cc, ot * QB:(ot + 1) * QB],
                            start=(cc == 0), stop=(cc == 1),
                            skip_group_check=True)
                    o_sb = sbpool.tile([128, QB], f32, tag="osb")
                    nc.vector.tensor_copy(o_sb[:], ps[:])
                    nc.sync.dma_start(
                        out[sc * 128:(sc + 1) * 128, ot * QB:(ot + 1) * QB],
                        o_sb[:])

                filler = []

                def emit_group(qb, hp):
                    qsl = slice(qb * QB, (qb + 1) * QB)
                    ctx = [ctp.tile([128, QB], f32, tag="ctxps",
                                    name=f"ctx{hh}") for hh in range(2)]
                    sts, pts = {}, {}

                    def scores(kc):
                        st = stp.tile([128, 2, QB], f32, tag="stps")
                        sts[kc] = st
                        for hh in range(2):
                            p0 = hh * 64
                            nc.tensor.matmul(
                                st[:, hh, :],
                                kt[hp][p0:p0 + 64, kc * 128:(kc + 1) * 128],
                                qt[hp][p0:p0 + 64, qsl],
                                start=True, stop=True,
                                tile_position=(p0, 0))
                        ptt = ptp.tile([128, 2, QB], bf16, tag="pt")
                        pts[kc] = ptt
                        nc.scalar.activation(ptt[:], st[:], Exp)

                    def pv(kc):
                        ptt = pts.pop(kc)
                        sts.pop(kc)
                        for hh in range(2):
                            nc.tensor.matmul(
                                ctx[hh][0:HD + 1, :],
                                v_all[:, kc, 2 * hp + hh, 0:HD + 1],
                                ptt[:, hh, :],
                                start=(kc == 0), stop=(kc == NKC - 1))

                    scores(0)
                    scores(1)
                    for kc in range(NKC):
                        if kc + 2 < NKC:
                            scores(kc + 2)
                        pv(kc)
                        if kc in (4, 7, 10, 13) and filler:
                            sc, ot = filler.pop(0)
                            emit_outproj(sc, ot, opp, ob)

                    # blend: ctxt = (PV * 0.6/r) + 0.4LV. First copy ctx off
                    # PSUM so the bank frees ~1us after pv(15) and the next
                    # group's accumulation never stalls on this chain.
                    for hh in range(2):
                        cs = sp.tile([65, QB], f32, tag="cs")
                        nc.vector.tensor_copy(cs[:], ctx[hh][0:HD + 1, :])
                        # partition 64 -> 0 move on ScalarE (tiny; Copy needs
                        # no act table so the Exp table stays resident)
                        bcs = sp.tile([1, QB], f32, tag="bcs")
                        nc.scalar.activation(
                            bcs[0:1, :], cs[64:65, :],
                            mybir.ActivationFunctionType.Copy)
                        rc = sp.tile([1, QB], f32, tag="rc")
                        nc.vector.reciprocal_approx_fast(rc[:], bcs[:])
                        bc = sp.tile([64, QB], f32, tag="bc")
                        nc.gpsimd.partition_broadcast(
                            bc[:], rc[0:1, :], channels=HD)
                        m1 = sp.tile([64, QB], f32, tag="m1")
                        nc.vector.tensor_mul(m1[:], cs[0:HD, :], bc[:])
                        if hh == 0:
                            nc.vector.scalar_tensor_tensor(
                                ctxt_all[0:64, hp, qsl], m1[:], 0.6,
                                lv_sb[0:64, hp, qsl], op0=mult, op1=add)
                        else:
                            stg = sp.tile([64, QB], bf16, tag="stg")
                            nc.vector.scalar_tensor_tensor(
                                stg[:], m1[:], 0.6,
                                lv1_sb[:, hp, qsl], op0=mult, op1=add)
                            nc.sync.dma_start(
                                ctxt_all[64:128, hp, qsl], stg[:])

                # (qb, hp) order chosen so out-proj(qb) becomes injectable
                # 1-2 groups after both hp halves of qb have blended.
                groups = [(0, 0), (1, 0), (0, 1), (1, 1),
                          (2, 0), (3, 0), (2, 1), (3, 1)]
                push_after = {2: 0, 3: 1, 6: 2, 7: 3}
                for gi, (qb, hp) in enumerate(groups):
                    emit_group(qb, hp)
                    if gi in push_after:
                        qb_o = push_after[gi]
                        for sc in range(qb_o * 4, qb_o * 4 + 4):
                            for ot in range(2):
                                filler.append((sc, ot))

            # ---- phase D: out-projection tail (wider PSUM pool) ----
            with (
                tc.tile_pool(name="ops2", bufs=4, space="PSUM") as opp2,
                tc.tile_pool(name="osb2", bufs=4) as ob2,
            ):
                while filler:
                    sc, ot = filler.pop(0)
                    emit_outproj(sc, ot, opp2, ob2)

    nc.compile()
    return nc


def _get_program():
    if "nc" not in _CACHE:
        _CACHE["nc"] = _build_program()
    return _CACHE["nc"]


def _in_maps(x, Wq, bq, Wk, bk, Wv, bv, Wo):
    xT = [np.ascontiguousarray(x[b].T).astype(BF16) for b in range(2)]
    maps = []
    for c in range(8):
        b, hg = c // 4, c % 4
        hs, he = hg * CLOC, (hg + 1) * CLOC
        maps.append({
            "xt": xT[b],
            "wqt": np.ascontiguousarray(Wq[hs:he].T / F32(8.0)).astype(BF16),
            "wkt": np.ascontiguousarray(Wk[hs:he].T).astype(BF16),
            "wvt": np.ascontiguousarray(Wv[hs:he].T).astype(BF16),
            "bqc": np.ascontiguousarray((bq[hs:he] / F32(8.0)).reshape(2, 128).T),
            "bkc": np.ascontiguousarray(bk[hs:he].reshape(2, 128).T),
            "bvr": bv[hs:he][None, :].astype(BF16),
            "wot": np.ascontiguousarray(Wo[:, hs:he].T).astype(BF16),
            "ltt": _LT_UNIQ,
        })
    return maps


def _run(x, Wq, bq, Wk, bk, Wv, bv, Wo, bo, trace=False):
    from concourse.bass_utils import run_bass_kernel_spmd
    nc = _get_program()
    maps = _in_maps(np.asarray(x, F32), np.asarray(Wq, F32), np.asarray(bq, F32),
                    np.asarray(Wk, F32), np.asarray(bk, F32), np.asarray(Wv, F32),
                    np.asarray(bv, F32), np.asarray(Wo, F32))
    res = run_bass_kernel_spmd(nc, maps, list(range(8)), trace=trace)
    bo = np.asarray(bo, F32)
    outp = np.empty((2, S, D), F32)
    for b in range(2):
        acc = res.results[b * 4]["out"].astype(F32)
        for hg in range(1, 4):
            acc = acc + res.results[b * 4 + hg]["out"]
        outp[b] = acc + bo
    return outp, res


def kernel(x, Wq, bq, Wk, bk, Wv, bv, Wo, bo):
    outp, _ = _run(x, Wq, bq, Wk, bk, Wv, bv, Wo, bo, trace=False)
    return outp


def kernel_traced(**inputs):
    return _run(trace=True, **inputs)
